# revision 34
# baseline (speedup 1.0000x reference)
"""Trainium2 Bass kernel for block-tridiagonal whitening (AR(1) recurrence).

Math: w_t = (x_t - mean(x_t)) @ V0 - w_{t-1} @ (V1 @ V0),  w_{-1} = 0.

Host-side transforms:
  V0c = (I - 11^T/C) @ V0   (centering folded into V0)
  M   = -(V1 @ V0)          (combined recurrence matrix)
so  w_t = x_t @ V0c + w_{t-1} @ M.

||M||_2 ~ 0.05, so the recurrence forgets its past within a few steps: each
S-step time chunk runs independently after a J-step warm-up from a y-only
state (error ~ ||M||^J); all chunks of both groups advance in lockstep.

Precision ladder (gate 2e-2 max-rel, this lands ~8e-3): x, V0c, staging and
output fp16; scan state + M fp8e4 (the correction is ~||M|| ~ 5% of w, so
fp8's ~4% rounding contributes ~2.5e-3).

Design notes (all probed on hardware):
  - PE operands allow ONE free dim; s-major staging keeps each scan step's
    column set contiguous per (h,b) for the DVE add (688ns/512el measured
    vs 1.7us strided).
  - ONE DVE add per step: w_col = pm + y_col written in place (the state
    and the output are the same value).  The fp8 copy of the new state for
    the next DoubleRow matmul is derived by a GpSimd cast (fp16->fp8 is
    the one pattern GpSimd is fast at; it cannot touch PSUM so it could
    never have done the add itself).
  - Both groups' scans run interleaved with a stagger so the PE->DVE->GPS
    chain of one group hides behind the other group's slot; group 0
    finishes early enough that its staging dump overlaps group 1's tail.
  - No transpose on device at all: the s-major staging buffer is dumped
    raw to DRAM (two contiguous DMAs per batch row) and the host
    unscrambles.  This deleted the gather+TensorE-transpose+copy output
    path (~160us of engine time) of the previous revision.
  - y copy PSUM->s-major staging iterates chunk-inner: strided PSUM source
    is cheap, and the staging side becomes 16-el contiguous runs.

Sharding: batch 64 -> 8 cores x 8 rows; parameters replicated.
"""

import sys

sys.path.insert(0, "/opt/trn_rl_repo")

import numpy as np
import ml_dtypes

B, T, C = 64, 2048, 256
NCORES = 8
BS = B // NCORES   # batch rows per core
S = 32             # scan chunk length
J = 3              # warm-up steps (||M||^J ~ 1e-4 relative)
HALO = 32          # reserved halo columns (only last J used)
NG = 2             # time groups
TG = T // NG       # time steps per group
CHG = TG // S      # chunks per group per batch row
NSTEP = S + J      # scan steps per group
KSTAG = 11         # group-1 scan stagger (slots; covers y(g1) fill)
COLS_PAD = 33 * 32 # s-major grid: position(t'') = (t''%32)*33 + t''//32


def _build_program():
    import concourse.bacc as bacc
    import concourse.mybir as mybir
    import concourse.tile as tile

    f32 = mybir.dt.float32
    f16 = mybir.dt.float16
    f8 = mybir.dt.float8e4
    DR = mybir.MatmulPerfMode.DoubleRow

    nc = bacc.Bacc("TRN2", target_bir_lowering=False, debug=False)

    xh_dram = nc.dram_tensor("xh", [BS, T, C], f16, kind="ExternalInput")
    # raw s-major staging dump; the host unscrambles to [B, T, C]
    w_dram = nc.dram_tensor("w", [NG, 128, 2, BS, COLS_PAD], f16,
                            kind="ExternalOutput")
    # weight quadrants: q[p, kh, mh, j] = W[kh*128 + p, mh*128 + j]
    vq_dram = nc.dram_tensor("vq", [128, 2, 2, 128], f16, kind="ExternalInput")
    mq_dram = nc.dram_tensor("mq8", [128, 2, 2, 128], f8, kind="ExternalInput")

    with tile.TileContext(nc) as tc:
        with (
            tc.tile_pool(name="const", bufs=1) as cpool,
            tc.tile_pool(name="stage", bufs=1) as spool,
            tc.tile_pool(name="state", bufs=1) as stpool,
            tc.tile_pool(name="xload", bufs=16) as xpool,
            tc.tile_pool(name="py", bufs=4, space="PSUM") as py_pool,
            tc.tile_pool(name="ps", bufs=4, space="PSUM") as ps_pool,
        ):
            vq = cpool.tile([128, 2, 2, 128], f16)
            mq = cpool.tile([128, 2, 2, 128], f8)
            nc.sync.dma_start(vq[:], vq_dram.ap()[:])
            nc.sync.dma_start(mq[:], mq_dram.ap()[:])

            xw = [spool.tile([128, 2, BS, COLS_PAD], f16, tag=f"xw{g}",
                             name=f"xw{g}") for g in range(NG)]
            # [cq, s] view of the s-major grid (memory: pos = s*33 + cq)
            xwq = [xw[g][:].rearrange("p h b (s cq) -> p h b cq s", cq=33)
                   for g in range(NG)]
            # zero the J used halo columns of group 0 (t'' in [28, 32))
            nc.gpsimd.memset(
                xw[0][:].rearrange(
                    "p h b (s cq) -> p h b s cq", cq=33)[
                        :, :, :, HALO - J:HALO, 0], 0.0)

            # fp8 scan-state ping-pong tiles (written by GpSimd casts)
            sf = [[stpool.tile([128, 2, BS, CHG], f8, tag=f"sf{g}_{k}",
                               name=f"sf{g}_{k}") for k in range(2)]
                  for g in range(NG)]
            # fp16 scratch for warm-up steps: their w values must NOT land
            # in the staging buffer (they'd corrupt y columns the previous
            # chunk's true pass still reads)
            wsc = [stpool.tile([128, 2, BS, CHG], f16, tag=f"wsc{g}",
                               name=f"wsc{g}") for g in range(NG)]

            cp_y = [0]

            # ---- emission helpers ------------------------------------------
            def emit_x_dma(g, b):
                ht = xpool.tile([128, 2, TG], f16, tag="ht", name="ht")
                # transposing loads MUST be sync-issued: scalar-issued xbar
                # DMAs race with their PE consumers (measured corruption)
                for kh in range(2):
                    nc.sync.dma_start(
                        ht[:, kh, :],
                        xh_dram.ap()[b, g * TG:(g + 1) * TG,
                                     kh * 128:(kh + 1) * 128],
                        transpose=True)
                return ht

            def emit_y_unit(g, b, mh, ch, ht):
                pm = py_pool.tile([128, 512], f32, tag="pmy", name="pmy")
                sl = slice(ch * 512, ch * 512 + 512)
                for kh in range(2):
                    nc.tensor.matmul(
                        pm[:], vq[:, kh, mh, :], ht[:, kh, sl],
                        start=(kh == 0), stop=(kh == 1))
                # t'' = HALO + ch*512 + u, u = a*32 + s -> pos s*33 + cq0+a:
                # iterate chunk-inner for 16-el contiguous staging runs
                cq0 = 1 + ch * 16
                dst = xwq[g][:, mh, b, cq0:cq0 + 16, :].rearrange(
                    "p cq s -> p s cq")
                src = pm[:].rearrange("p (a s) -> p s a", s=32)
                # g0's copies split DVE/ACT (scan not running yet); g1's all
                # on ACT so DVE stays free for the scan adds+casts
                if g == 0 and cp_y[0] % 2 == 0:
                    nc.vector.tensor_copy(dst, src)
                else:
                    nc.scalar.copy(dst, src)
                cp_y[0] += 1

            def emit_y_dup(b):
                # seed group 1's halo from group 0's last J y-columns
                nc.gpsimd.tensor_copy(
                    xwq[1][:, :, b, 0, HALO - J:HALO],
                    xwq[0][:, :, b, 32, HALO - J:HALO])

            def col_slice(g, i):
                # columns {t'' = cc*32 + i + (HALO-J)} for cc in [0, CHG)
                tpp = i + HALO - J
                base = (tpp % 32) * 33 + tpp // 32
                return xw[g][:, :, :, base:base + CHG]

            def emit_scan_step(g, i):
                """One merged scan step: DR matmuls, one DVE add in place,
                fp8 state cast (split DVE/ACT by group) for the next step."""
                ys = col_slice(g, i)
                wdst = wsc[g][:] if i < J else ys
                if i > 0:
                    pm = ps_pool.tile([128, 2, BS, CHG], f32, tag="pms",
                                      name="pms")
                    prev = sf[g][(i - 1) % 2]
                    for mh in range(2):
                        nc.tensor.matmul(
                            pm[:, mh], mq[:, :, mh, :], prev[:],
                            start=True, stop=True, perf_mode=DR)
                    nc.vector.tensor_add(wdst, pm[:], ys)
                else:
                    wdst = ys
                if i < NSTEP - 1:
                    nc.vector.tensor_copy(sf[g][i % 2][:], wdst)

            def emit_dump(g, b):
                # issued off the load queue when possible; the dump has no
                # on-device consumer, so cross-queue issue is race-free
                eng = nc.scalar if (g * BS + b) % 2 == 0 else nc.sync
                eng.dma_start(w_dram.ap()[g, :, :, b, :],
                              xw[g][:, :, b, :])

            # ---- emission schedule -----------------------------------------
            # 1. all input DMAs up front (g0 first); y(g0) densely
            hts = {}
            for b in range(BS):
                hts[(0, b)] = emit_x_dma(0, b)
            for b in range(BS):
                hts[(1, b)] = emit_x_dma(1, b)
            for b in range(BS):
                for mh in range(2):
                    for ch in range(TG // 512):
                        emit_y_unit(0, b, mh, ch, hts[(0, b)])
                emit_y_dup(b)

            # 2. scan(g0) starts immediately; y(g1) woven into its early
            #    slots (copies only on ACT would stall; alternate as usual);
            #    scan(g1) staggered in once its y is done; dumps overlap
            y1q = [(b, mh, ch) for b in range(BS)
                   for mh in range(2) for ch in range(TG // 512)]
            NSLOT = NSTEP + KSTAG
            qi = 0
            di = 0
            for tau in range(NSLOT):
                if tau < NSTEP:
                    emit_scan_step(0, tau)
                # all y(g1) units MUST be emitted before scan(g1) starts:
                # emission order is the static schedule's dependency order
                for _ in range(3):
                    if qi < len(y1q):
                        b, mh, ch = y1q[qi]
                        qi += 1
                        emit_y_unit(1, b, mh, ch, hts[(1, b)])
                if KSTAG <= tau:
                    assert qi >= len(y1q)
                    emit_scan_step(1, tau - KSTAG)
                if tau > NSTEP and di < BS:
                    emit_dump(0, di)
                    di += 1
            while di < BS:
                emit_dump(0, di)
                di += 1

            # 3. dump(g1)
            for b in range(BS):
                emit_dump(1, b)

    nc.compile()
    return nc


_NC_CACHE = None


def _prep_inputs(x, V_0, V_1):
    x = np.ascontiguousarray(np.asarray(x, dtype=np.float32))
    V0 = np.asarray(V_0, dtype=np.float64)
    V1 = np.asarray(V_1, dtype=np.float64)

    P = np.eye(C) - 1.0 / C
    V0c = (P @ V0).astype(np.float32)
    M = (-(V1 @ V0)).astype(np.float32)

    x_h = x.astype(np.float16)
    V_h = V0c.astype(np.float16)
    M_8 = M.astype(ml_dtypes.float8_e4m3)

    def quads(w):
        return np.ascontiguousarray(
            w.reshape(2, 128, 2, 128).transpose(1, 0, 2, 3))

    return x_h, quads(V_h), quads(M_8)


def _unscramble(dump):
    """[NG, 128, 2, BS, COLS_PAD] s-major staging dump -> [BS, T, C]."""
    tpp = np.arange(TG) + HALO
    pos = (tpp % 32) * 33 + tpp // 32
    out = np.empty((BS, T, C), dtype=np.float16)
    for g in range(NG):
        sel = dump[g][:, :, :, pos]            # [128, 2, BS, TG]
        out[:, g * TG:(g + 1) * TG, :] = (
            sel.transpose(2, 3, 1, 0).reshape(BS, TG, C))
    return out


def kernel(x, V_0, V_1):
    global _NC_CACHE
    from concourse.bass_utils import run_bass_kernel_spmd

    x_h, vq, mq8 = _prep_inputs(x, V_0, V_1)

    if _NC_CACHE is None:
        _NC_CACHE = _build_program()
    nc = _NC_CACHE

    in_maps = []
    for core in range(NCORES):
        sl = slice(core * BS, (core + 1) * BS)
        in_maps.append({
            "xh": np.ascontiguousarray(x_h[sl]),
            "vq": vq, "mq8": mq8,
        })

    res = run_bass_kernel_spmd(nc, in_maps, core_ids=list(range(NCORES)))
    out = np.concatenate(
        [_unscramble(np.asarray(res.results[i]["w"]))
         for i in range(NCORES)], axis=0)
    return out.astype(np.float32)


# revision 36
# speedup vs baseline: 1.0099x; 1.0099x over previous
"""Trainium2 Bass kernel for block-tridiagonal whitening (AR(1) recurrence).

Math: w_t = (x_t - mean(x_t)) @ V0 - w_{t-1} @ (V1 @ V0),  w_{-1} = 0.

Host-side transforms:
  V0c = (I - 11^T/C) @ V0   (centering folded into V0)
  M   = -(V1 @ V0)          (combined recurrence matrix)
so  w_t = x_t @ V0c + w_{t-1} @ M.

||M||_2 ~ 0.05, so the recurrence forgets its past within a few steps: each
S-step time chunk runs independently after a J-step warm-up from a y-only
state (error ~ ||M||^J); all chunks of both groups advance in lockstep.

Precision ladder (gate 2e-2 max-rel, this lands ~8e-3): x, V0c, staging and
output fp16; scan state + M fp8e4 (the correction is ~||M|| ~ 5% of w, so
fp8's ~4% rounding contributes ~2.5e-3).

Design notes (all probed on hardware):
  - PE operands allow ONE free dim; s-major staging keeps each scan step's
    column set contiguous per (h,b) for the DVE add (688ns/512el measured
    vs 1.7us strided).
  - ONE DVE add per step: w_col = pm + y_col written in place (the state
    and the output are the same value).  The fp8 copy of the new state for
    the next DoubleRow matmul is derived by a GpSimd cast (fp16->fp8 is
    the one pattern GpSimd is fast at; it cannot touch PSUM so it could
    never have done the add itself).
  - Both groups' scans run interleaved with a stagger so the PE->DVE->GPS
    chain of one group hides behind the other group's slot; group 0
    finishes early enough that its staging dump overlaps group 1's tail.
  - No transpose on device at all: the s-major staging buffer is dumped
    raw to DRAM (two contiguous DMAs per batch row) and the host
    unscrambles.  This deleted the gather+TensorE-transpose+copy output
    path (~160us of engine time) of the previous revision.
  - y copy PSUM->s-major staging iterates chunk-inner: strided PSUM source
    is cheap, and the staging side becomes 16-el contiguous runs.

Sharding: batch 64 -> 8 cores x 8 rows; parameters replicated.
"""

import sys

sys.path.insert(0, "/opt/trn_rl_repo")

import numpy as np
import ml_dtypes

B, T, C = 64, 2048, 256
NCORES = 8
BS = B // NCORES   # batch rows per core
S = 32             # scan chunk length
J = 3              # warm-up steps (||M||^J ~ 1e-4 relative)
HALO = 32          # reserved halo columns (only last J used)
NG = 2             # time groups
TG = T // NG       # time steps per group
CHG = TG // S      # chunks per group per batch row
NSTEP = S + J      # scan steps per group
KSTAG = 10         # group-1 scan stagger (slots; covers y(g1) fill)
COLS_PAD = 33 * 32 # s-major grid: position(t'') = (t''%32)*33 + t''//32


def _build_program():
    import concourse.bacc as bacc
    import concourse.mybir as mybir
    import concourse.tile as tile

    f32 = mybir.dt.float32
    f16 = mybir.dt.float16
    f8 = mybir.dt.float8e4
    DR = mybir.MatmulPerfMode.DoubleRow

    nc = bacc.Bacc("TRN2", target_bir_lowering=False, debug=False)

    xh_dram = nc.dram_tensor("xh", [BS, T, C], f16, kind="ExternalInput")
    # raw s-major staging dump; the host unscrambles to [B, T, C]
    w_dram = nc.dram_tensor("w", [NG, 128, 2, BS, COLS_PAD], f16,
                            kind="ExternalOutput")
    # weight quadrants: q[p, kh, mh, j] = W[kh*128 + p, mh*128 + j]
    vq_dram = nc.dram_tensor("vq", [128, 2, 2, 128], f16, kind="ExternalInput")
    mq_dram = nc.dram_tensor("mq8", [128, 2, 2, 128], f8, kind="ExternalInput")

    with tile.TileContext(nc) as tc:
        with (
            tc.tile_pool(name="const", bufs=1) as cpool,
            tc.tile_pool(name="stage", bufs=1) as spool,
            tc.tile_pool(name="state", bufs=1) as stpool,
            tc.tile_pool(name="xload", bufs=16) as xpool,
            tc.tile_pool(name="py", bufs=4, space="PSUM") as py_pool,
            tc.tile_pool(name="ps", bufs=4, space="PSUM") as ps_pool,
        ):
            vq = cpool.tile([128, 2, 2, 128], f16)
            mq = cpool.tile([128, 2, 2, 128], f8)
            nc.sync.dma_start(vq[:], vq_dram.ap()[:])
            nc.sync.dma_start(mq[:], mq_dram.ap()[:])

            xw = [spool.tile([128, 2, BS, COLS_PAD], f16, tag=f"xw{g}",
                             name=f"xw{g}") for g in range(NG)]
            # [cq, s] view of the s-major grid (memory: pos = s*33 + cq)
            xwq = [xw[g][:].rearrange("p h b (s cq) -> p h b cq s", cq=33)
                   for g in range(NG)]
            # zero the J used halo columns of group 0 (t'' in [28, 32))
            nc.gpsimd.memset(
                xw[0][:].rearrange(
                    "p h b (s cq) -> p h b s cq", cq=33)[
                        :, :, :, HALO - J:HALO, 0], 0.0)

            # fp8 scan-state ping-pong tiles (written by GpSimd casts)
            sf = [[stpool.tile([128, 2, BS, CHG], f8, tag=f"sf{g}_{k}",
                               name=f"sf{g}_{k}") for k in range(2)]
                  for g in range(NG)]
            # fp16 scratch for warm-up steps: their w values must NOT land
            # in the staging buffer (they'd corrupt y columns the previous
            # chunk's true pass still reads)
            wsc = [stpool.tile([128, 2, BS, CHG], f16, tag=f"wsc{g}",
                               name=f"wsc{g}") for g in range(NG)]

            cp_y = [0]

            # ---- emission helpers ------------------------------------------
            def emit_x_dma(g, b):
                ht = xpool.tile([128, 2, TG], f16, tag="ht", name="ht")
                # transposing loads MUST be sync-issued: scalar-issued xbar
                # DMAs race with their PE consumers (measured corruption)
                for kh in range(2):
                    nc.sync.dma_start(
                        ht[:, kh, :],
                        xh_dram.ap()[b, g * TG:(g + 1) * TG,
                                     kh * 128:(kh + 1) * 128],
                        transpose=True)
                return ht

            def emit_y_unit(g, b, mh, ch, ht):
                pm = py_pool.tile([128, 512], f32, tag="pmy", name="pmy")
                sl = slice(ch * 512, ch * 512 + 512)
                for kh in range(2):
                    nc.tensor.matmul(
                        pm[:], vq[:, kh, mh, :], ht[:, kh, sl],
                        start=(kh == 0), stop=(kh == 1))
                # t'' = HALO + ch*512 + u, u = a*32 + s -> pos s*33 + cq0+a:
                # iterate chunk-inner for 16-el contiguous staging runs
                cq0 = 1 + ch * 16
                dst = xwq[g][:, mh, b, cq0:cq0 + 16, :].rearrange(
                    "p cq s -> p s cq")
                src = pm[:].rearrange("p (a s) -> p s a", s=32)
                # g0's copies split DVE/ACT (scan not running yet); g1's all
                # on ACT so DVE stays free for the scan adds+casts
                if g == 0 and cp_y[0] % 2 == 0:
                    nc.vector.tensor_copy(dst, src)
                else:
                    nc.scalar.copy(dst, src)
                cp_y[0] += 1

            def emit_y_dup(b):
                # seed group 1's halo from group 0's last J y-columns
                nc.gpsimd.tensor_copy(
                    xwq[1][:, :, b, 0, HALO - J:HALO],
                    xwq[0][:, :, b, 32, HALO - J:HALO])

            def col_slice(g, i):
                # columns {t'' = cc*32 + i + (HALO-J)} for cc in [0, CHG)
                tpp = i + HALO - J
                base = (tpp % 32) * 33 + tpp // 32
                return xw[g][:, :, :, base:base + CHG]

            def emit_scan_step(g, i):
                """One merged scan step: DR matmuls, one DVE add in place,
                fp8 state cast (split DVE/ACT by group) for the next step."""
                ys = col_slice(g, i)
                wdst = wsc[g][:] if i < J else ys
                if i > 0:
                    pm = ps_pool.tile([128, 2, BS, CHG], f32, tag="pms",
                                      name="pms")
                    prev = sf[g][(i - 1) % 2]
                    for mh in range(2):
                        nc.tensor.matmul(
                            pm[:, mh], mq[:, :, mh, :], prev[:],
                            start=True, stop=True, perf_mode=DR)
                    nc.vector.tensor_add(wdst, pm[:], ys)
                else:
                    wdst = ys
                if i < NSTEP - 1:
                    nc.vector.tensor_copy(sf[g][i % 2][:], wdst)

            def emit_dump(g, b):
                # issued off the load queue when possible; the dump has no
                # on-device consumer, so cross-queue issue is race-free
                eng = nc.scalar if (g * BS + b) % 2 == 0 else nc.sync
                eng.dma_start(w_dram.ap()[g, :, :, b, :],
                              xw[g][:, :, b, :])

            # ---- emission schedule -----------------------------------------
            # 1. all input DMAs up front (g0 first); y(g0) densely
            hts = {}
            for b in range(BS):
                hts[(0, b)] = emit_x_dma(0, b)
            for b in range(BS):
                hts[(1, b)] = emit_x_dma(1, b)
            for b in range(BS):
                for mh in range(2):
                    for ch in range(TG // 512):
                        emit_y_unit(0, b, mh, ch, hts[(0, b)])
                emit_y_dup(b)

            # 2. scan(g0) starts immediately; y(g1) woven into its early
            #    slots (copies only on ACT would stall; alternate as usual);
            #    scan(g1) staggered in once its y is done; dumps overlap
            y1q = [(b, mh, ch) for b in range(BS)
                   for mh in range(2) for ch in range(TG // 512)]
            NSLOT = NSTEP + KSTAG
            qi = 0
            di = 0
            for tau in range(NSLOT):
                if tau < NSTEP:
                    emit_scan_step(0, tau)
                # all y(g1) units MUST be emitted before scan(g1) starts:
                # emission order is the static schedule's dependency order
                for _ in range(4):
                    if qi < len(y1q):
                        b, mh, ch = y1q[qi]
                        qi += 1
                        emit_y_unit(1, b, mh, ch, hts[(1, b)])
                if KSTAG <= tau:
                    assert qi >= len(y1q)
                    emit_scan_step(1, tau - KSTAG)
                if tau > NSTEP and di < BS:
                    emit_dump(0, di)
                    di += 1
            while di < BS:
                emit_dump(0, di)
                di += 1

            # 3. dump(g1)
            for b in range(BS):
                emit_dump(1, b)

    nc.compile()
    return nc


_NC_CACHE = None


def _prep_inputs(x, V_0, V_1):
    x = np.ascontiguousarray(np.asarray(x, dtype=np.float32))
    V0 = np.asarray(V_0, dtype=np.float64)
    V1 = np.asarray(V_1, dtype=np.float64)

    P = np.eye(C) - 1.0 / C
    V0c = (P @ V0).astype(np.float32)
    M = (-(V1 @ V0)).astype(np.float32)

    x_h = x.astype(np.float16)
    V_h = V0c.astype(np.float16)
    M_8 = M.astype(ml_dtypes.float8_e4m3)

    def quads(w):
        return np.ascontiguousarray(
            w.reshape(2, 128, 2, 128).transpose(1, 0, 2, 3))

    return x_h, quads(V_h), quads(M_8)


def _unscramble(dump):
    """[NG, 128, 2, BS, COLS_PAD] s-major staging dump -> [BS, T, C]."""
    tpp = np.arange(TG) + HALO
    pos = (tpp % 32) * 33 + tpp // 32
    out = np.empty((BS, T, C), dtype=np.float16)
    for g in range(NG):
        sel = dump[g][:, :, :, pos]            # [128, 2, BS, TG]
        out[:, g * TG:(g + 1) * TG, :] = (
            sel.transpose(2, 3, 1, 0).reshape(BS, TG, C))
    return out


def kernel(x, V_0, V_1):
    global _NC_CACHE
    from concourse.bass_utils import run_bass_kernel_spmd

    x_h, vq, mq8 = _prep_inputs(x, V_0, V_1)

    if _NC_CACHE is None:
        _NC_CACHE = _build_program()
    nc = _NC_CACHE

    in_maps = []
    for core in range(NCORES):
        sl = slice(core * BS, (core + 1) * BS)
        in_maps.append({
            "xh": np.ascontiguousarray(x_h[sl]),
            "vq": vq, "mq8": mq8,
        })

    res = run_bass_kernel_spmd(nc, in_maps, core_ids=list(range(NCORES)))
    out = np.concatenate(
        [_unscramble(np.asarray(res.results[i]["w"]))
         for i in range(NCORES)], axis=0)
    return out.astype(np.float32)


# revision 37
# speedup vs baseline: 1.0414x; 1.0313x over previous
"""Trainium2 Bass kernel for block-tridiagonal whitening (AR(1) recurrence).

Math: w_t = (x_t - mean(x_t)) @ V0 - w_{t-1} @ (V1 @ V0),  w_{-1} = 0.

Host-side transforms:
  V0c = (I - 11^T/C) @ V0   (centering folded into V0)
  M   = -(V1 @ V0)          (combined recurrence matrix)
so  w_t = x_t @ V0c + w_{t-1} @ M.

||M||_2 ~ 0.05, so the recurrence forgets its past within a few steps: each
S-step time chunk runs independently after a J-step warm-up from a y-only
state (error ~ ||M||^J); all chunks of both groups advance in lockstep.

Precision ladder (gate 2e-2 max-rel, this lands ~8e-3): x, V0c, staging and
output fp16; scan state + M fp8e4 (the correction is ~||M|| ~ 5% of w, so
fp8's ~4% rounding contributes ~2.5e-3).

Design notes (all probed on hardware):
  - PE operands allow ONE free dim; s-major staging keeps each scan step's
    column set contiguous per (h,b) for the DVE add (688ns/512el measured
    vs 1.7us strided).
  - ONE DVE add per step: w_col = pm + y_col written in place (the state
    and the output are the same value).  The fp8 copy of the new state for
    the next DoubleRow matmul is derived by a GpSimd cast (fp16->fp8 is
    the one pattern GpSimd is fast at; it cannot touch PSUM so it could
    never have done the add itself).
  - Both groups' scans run interleaved with a stagger so the PE->DVE->GPS
    chain of one group hides behind the other group's slot; group 0
    finishes early enough that its staging dump overlaps group 1's tail.
  - No transpose on device at all: the s-major staging buffer is dumped
    raw to DRAM (two contiguous DMAs per batch row) and the host
    unscrambles.  This deleted the gather+TensorE-transpose+copy output
    path (~160us of engine time) of the previous revision.
  - y copy PSUM->s-major staging iterates chunk-inner: strided PSUM source
    is cheap, and the staging side becomes 16-el contiguous runs.

Sharding: batch 64 -> 8 cores x 8 rows; parameters replicated.
"""

import sys

sys.path.insert(0, "/opt/trn_rl_repo")

import numpy as np
import ml_dtypes

B, T, C = 64, 2048, 256
NCORES = 8
BS = B // NCORES   # batch rows per core
S = 32             # scan chunk length
J = 4              # warm-up steps (||M||^J ~ 6e-6 relative)
HALO = 32          # reserved halo columns (only last J used)
NG = 2             # time groups
TG = T // NG       # time steps per group
CHG = TG // S      # chunks per group per batch row
NSTEP = S + J      # scan steps per group
KSTAG = 10         # group-1 scan stagger (slots; covers y(g1) fill)
COLS_PAD = 33 * 32 # s-major grid: position(t'') = (t''%32)*33 + t''//32


def _build_program():
    import concourse.bacc as bacc
    import concourse.mybir as mybir
    import concourse.tile as tile

    f32 = mybir.dt.float32
    f16 = mybir.dt.float16
    f8 = mybir.dt.float8e4
    DR = mybir.MatmulPerfMode.DoubleRow

    nc = bacc.Bacc("TRN2", target_bir_lowering=False, debug=False)

    xh_dram = nc.dram_tensor("xh", [BS, T, C], f16, kind="ExternalInput")
    # raw s-major staging dump; the host unscrambles to [B, T, C]
    w_dram = nc.dram_tensor("w", [NG, 128, 2, BS, COLS_PAD], f16,
                            kind="ExternalOutput")
    # weight quadrants: q[p, kh, mh, j] = W[kh*128 + p, mh*128 + j]
    vq_dram = nc.dram_tensor("vq", [128, 2, 2, 128], f16, kind="ExternalInput")
    mq_dram = nc.dram_tensor("mq8", [128, 2, 2, 128], f8, kind="ExternalInput")

    with tile.TileContext(nc) as tc:
        with (
            tc.tile_pool(name="const", bufs=1) as cpool,
            tc.tile_pool(name="stage", bufs=1) as spool,
            tc.tile_pool(name="state", bufs=1) as stpool,
            tc.tile_pool(name="xload", bufs=16) as xpool,
            tc.tile_pool(name="py", bufs=4, space="PSUM") as py_pool,
            tc.tile_pool(name="ps", bufs=4, space="PSUM") as ps_pool,
        ):
            vq = cpool.tile([128, 2, 2, 128], f16)
            mq = cpool.tile([128, 2, 2, 128], f8)
            nc.sync.dma_start(vq[:], vq_dram.ap()[:])
            nc.sync.dma_start(mq[:], mq_dram.ap()[:])

            xw = [spool.tile([128, 2, BS, COLS_PAD], f16, tag=f"xw{g}",
                             name=f"xw{g}") for g in range(NG)]
            # [cq, s] view of the s-major grid (memory: pos = s*33 + cq)
            xwq = [xw[g][:].rearrange("p h b (s cq) -> p h b cq s", cq=33)
                   for g in range(NG)]
            # zero the J used halo columns of group 0 (t'' in [28, 32))
            nc.gpsimd.memset(
                xw[0][:].rearrange(
                    "p h b (s cq) -> p h b s cq", cq=33)[
                        :, :, :, HALO - J:HALO, 0], 0.0)

            # fp8 scan-state ping-pong tiles (written by GpSimd casts)
            sf = [[stpool.tile([128, 2, BS, CHG], f8, tag=f"sf{g}_{k}",
                               name=f"sf{g}_{k}") for k in range(2)]
                  for g in range(NG)]
            # fp16 scratch for warm-up steps: their w values must NOT land
            # in the staging buffer (they'd corrupt y columns the previous
            # chunk's true pass still reads)
            wsc = [stpool.tile([128, 2, BS, CHG], f16, tag=f"wsc{g}",
                               name=f"wsc{g}") for g in range(NG)]

            cp_y = [0]

            # ---- emission helpers ------------------------------------------
            def emit_x_dma(g, b):
                ht = xpool.tile([128, 2, TG], f16, tag="ht", name="ht")
                # transposing loads MUST be sync-issued: scalar-issued xbar
                # DMAs race with their PE consumers (measured corruption)
                for kh in range(2):
                    nc.sync.dma_start(
                        ht[:, kh, :],
                        xh_dram.ap()[b, g * TG:(g + 1) * TG,
                                     kh * 128:(kh + 1) * 128],
                        transpose=True)
                return ht

            def emit_y_unit(g, b, mh, ch, ht):
                pm = py_pool.tile([128, 512], f32, tag="pmy", name="pmy")
                sl = slice(ch * 512, ch * 512 + 512)
                for kh in range(2):
                    nc.tensor.matmul(
                        pm[:], vq[:, kh, mh, :], ht[:, kh, sl],
                        start=(kh == 0), stop=(kh == 1))
                # t'' = HALO + ch*512 + u, u = a*32 + s -> pos s*33 + cq0+a:
                # iterate chunk-inner for 16-el contiguous staging runs
                cq0 = 1 + ch * 16
                dst = xwq[g][:, mh, b, cq0:cq0 + 16, :].rearrange(
                    "p cq s -> p s cq")
                src = pm[:].rearrange("p (a s) -> p s a", s=32)
                # g0's copies split DVE/ACT (scan not running yet); g1's all
                # on ACT so DVE stays free for the scan adds+casts
                if g == 0 and cp_y[0] % 2 == 0:
                    nc.vector.tensor_copy(dst, src)
                else:
                    nc.scalar.copy(dst, src)
                cp_y[0] += 1

            def emit_y_dup(b):
                # seed group 1's halo from group 0's last J y-columns
                nc.gpsimd.tensor_copy(
                    xwq[1][:, :, b, 0, HALO - J:HALO],
                    xwq[0][:, :, b, 32, HALO - J:HALO])

            def col_slice(g, i):
                # columns {t'' = cc*32 + i + (HALO-J)} for cc in [0, CHG)
                tpp = i + HALO - J
                base = (tpp % 32) * 33 + tpp // 32
                return xw[g][:, :, :, base:base + CHG]

            def emit_scan_step(g, i):
                """One merged scan step: DR matmuls, one DVE add in place,
                fp8 state cast (split DVE/ACT by group) for the next step."""
                ys = col_slice(g, i)
                wdst = wsc[g][:] if i < J else ys
                if i > 0:
                    pm = ps_pool.tile([128, 2, BS, CHG], f32, tag="pms",
                                      name="pms")
                    prev = sf[g][(i - 1) % 2]
                    for mh in range(2):
                        nc.tensor.matmul(
                            pm[:, mh], mq[:, :, mh, :], prev[:],
                            start=True, stop=True, perf_mode=DR)
                    nc.vector.tensor_add(wdst, pm[:], ys)
                else:
                    wdst = ys
                if i < NSTEP - 1:
                    nc.vector.tensor_copy(sf[g][i % 2][:], wdst)

            def emit_dump(g, b):
                # issued off the load queue when possible; the dump has no
                # on-device consumer, so cross-queue issue is race-free
                eng = nc.scalar if (g * BS + b) % 2 == 0 else nc.sync
                eng.dma_start(w_dram.ap()[g, :, :, b, :],
                              xw[g][:, :, b, :])

            # ---- emission schedule -----------------------------------------
            # 1. all input DMAs up front (g0 first); y(g0) densely
            hts = {}
            for b in range(BS):
                hts[(0, b)] = emit_x_dma(0, b)
            for b in range(BS):
                hts[(1, b)] = emit_x_dma(1, b)
            for b in range(BS):
                for mh in range(2):
                    for ch in range(TG // 512):
                        emit_y_unit(0, b, mh, ch, hts[(0, b)])
                emit_y_dup(b)

            # 2. scan(g0) starts immediately; y(g1) woven into its early
            #    slots (copies only on ACT would stall; alternate as usual);
            #    scan(g1) staggered in once its y is done; dumps overlap
            y1q = [(b, mh, ch) for b in range(BS)
                   for mh in range(2) for ch in range(TG // 512)]
            NSLOT = NSTEP + KSTAG
            qi = 0
            di = 0
            for tau in range(NSLOT):
                if tau < NSTEP:
                    emit_scan_step(0, tau)
                # all y(g1) units MUST be emitted before scan(g1) starts:
                # emission order is the static schedule's dependency order
                for _ in range(4):
                    if qi < len(y1q):
                        b, mh, ch = y1q[qi]
                        qi += 1
                        emit_y_unit(1, b, mh, ch, hts[(1, b)])
                if KSTAG <= tau:
                    assert qi >= len(y1q)
                    emit_scan_step(1, tau - KSTAG)
                if tau > NSTEP and di < BS:
                    emit_dump(0, di)
                    di += 1
            while di < BS:
                emit_dump(0, di)
                di += 1

            # 3. dump(g1)
            for b in range(BS):
                emit_dump(1, b)

    nc.compile()
    return nc


_NC_CACHE = None


def _prep_inputs(x, V_0, V_1):
    x = np.ascontiguousarray(np.asarray(x, dtype=np.float32))
    V0 = np.asarray(V_0, dtype=np.float64)
    V1 = np.asarray(V_1, dtype=np.float64)

    P = np.eye(C) - 1.0 / C
    V0c = (P @ V0).astype(np.float32)
    M = (-(V1 @ V0)).astype(np.float32)

    x_h = x.astype(np.float16)
    V_h = V0c.astype(np.float16)
    M_8 = M.astype(ml_dtypes.float8_e4m3)

    def quads(w):
        return np.ascontiguousarray(
            w.reshape(2, 128, 2, 128).transpose(1, 0, 2, 3))

    return x_h, quads(V_h), quads(M_8)


def _unscramble(dump):
    """[NG, 128, 2, BS, COLS_PAD] s-major staging dump -> [BS, T, C]."""
    tpp = np.arange(TG) + HALO
    pos = (tpp % 32) * 33 + tpp // 32
    out = np.empty((BS, T, C), dtype=np.float16)
    for g in range(NG):
        sel = dump[g][:, :, :, pos]            # [128, 2, BS, TG]
        out[:, g * TG:(g + 1) * TG, :] = (
            sel.transpose(2, 3, 1, 0).reshape(BS, TG, C))
    return out


def kernel(x, V_0, V_1):
    global _NC_CACHE
    from concourse.bass_utils import run_bass_kernel_spmd

    x_h, vq, mq8 = _prep_inputs(x, V_0, V_1)

    if _NC_CACHE is None:
        _NC_CACHE = _build_program()
    nc = _NC_CACHE

    in_maps = []
    for core in range(NCORES):
        sl = slice(core * BS, (core + 1) * BS)
        in_maps.append({
            "xh": np.ascontiguousarray(x_h[sl]),
            "vq": vq, "mq8": mq8,
        })

    res = run_bass_kernel_spmd(nc, in_maps, core_ids=list(range(NCORES)))
    out = np.concatenate(
        [_unscramble(np.asarray(res.results[i]["w"]))
         for i in range(NCORES)], axis=0)
    return out.astype(np.float32)
